# revision 22
# baseline (speedup 1.0000x reference)
"""Multi-head attention (B=2, L=S=2048, D=1024, H=16) on 8 Trainium2 cores.

Sharding: core c -> batch b = c // 4, head group g = c % 4 (4 heads per core).
W_Q/K/V column-sharded (256 cols per core), W_O row-sharded (256 rows per core);
the 4 partial outputs per batch are summed on the host (plus bias terms).

v5: the scalar-engine exp paces the attention loop (2 x ~1.04us per
iteration); the scores double-buffer in PSUM ("sc" pool, 2 slots) creates a
write-after-read chain from each iteration's first scores matmul to a
previous exp.  Scheduling rules used here:
  - the score-pair emission order alternates every iteration, so the next
    iteration's first scores always land on the slot that was read earliest;
  - any other PSUM tile injected into the rotation (projection pieces,
    output-projection tiles) is injected in pairs of 1-bank tiles so the
    rotation parity is preserved;
  - AV runs two iterations behind (AV(st-2) after scores/exp/mask(st)), so
    the T accumulator banks are only needed from iteration 2 of each l-tile,
    which lets the epilogue of the previous l-tile run straight out of PSUM
    during iterations 0-1 (no drain copies at all);
  - xT is 8 separate tiles so the first V/KT/QT matmuls start on the first
    DMA chunk; V + KT chunk0 + QT0 are emitted kd-interleaved, racing the
    chunk DMAs; ~2.5us of warm-up matmuls lift the PE clock gate (HAM) to
    2.4 GHz before the real work lands;
  - KT chunks 1-3 / QT1-3 are injected as N=256 piece-pairs inside the loop;
    the output projection of l-tile t runs during l-tile t+1; output DMAs
    alternate between the sync and gpsimd queues.

Math per core (all matmul operands fp16, PSUM fp32):
  QT = 0.125*(x Wq + bq)^T, KT = (x Wk + bk)^T  (feature-major [256, L]);
  Vaug = [V_h | ones] (even h) / [ones | V_h] (odd h); bv folded out on the
  host (softmax rows sum to 1 => + bv @ Wo + bo once).
  S^T = KT^T QT per (pair, 128-key tile, 512-l tile) -- the two K=64 matmuls
  of a pair occupy disjoint PE row-groups and overlap; E = exp(S^T) * maskT;
  T_h += Vaug_h^T E (head output + softmax row-sums in one matmul);
  outT = T_av * recip(T_sum); out_partial = outT^T Wo_rows.
"""
from contextlib import ExitStack

import numpy as np

import concourse.bass as bass
import concourse.mybir as mybir
import concourse.tile as tile
from concourse import bacc
from concourse.bass_utils import run_bass_kernel_spmd

F16 = mybir.dt.float16
F32 = mybir.dt.float32

D = 1024          # d_model
H = 16            # heads
DK = 64           # head dim
B, L = 2, 2048
NCORES = 8
HPC = 4           # heads per core
FPC = HPC * DK    # features per core = 256
KD = D // 128     # 8 contraction subtiles for projections
LT, LTW = 4, 512  # l tiles
ST, STW = 16, 128  # s tiles
Ident = mybir.ActivationFunctionType.Identity
Exp = mybir.ActivationFunctionType.Exp
MULT = mybir.AluOpType.mult
ADD = mybir.AluOpType.add

_CACHED_NC = None


def _build():
    nc = bacc.Bacc("TRN2", target_bir_lowering=False, debug=False,
                   num_devices=NCORES)
    xT = nc.declare_dram_parameter("xT", [128, KD, L], F16, isOutput=False)
    wq = nc.declare_dram_parameter("wq", [128, KD, FPC], F16, isOutput=False)
    wk = nc.declare_dram_parameter("wk", [128, KD, FPC], F16, isOutput=False)
    wv = nc.declare_dram_parameter("wv", [128, KD, FPC], F16, isOutput=False)
    wo = nc.declare_dram_parameter("wo", [128, 2, D], F16, isOutput=False)
    bq = nc.declare_dram_parameter("bq", [128, 2], F32, isOutput=False)
    bk = nc.declare_dram_parameter("bk", [128, 2], F32, isOutput=False)
    maskT = nc.declare_dram_parameter("maskT", [ST, LT, 128, LTW], F16,
                                      isOutput=False)
    out = nc.declare_dram_parameter("out", [128, ST, 2, LTW], F16,
                                    isOutput=True)

    with tile.TileContext(nc) as tc, ExitStack() as ctx:
        pool = ctx.enter_context(tc.tile_pool(name="pers", bufs=1))
        mpool = ctx.enter_context(tc.tile_pool(name="mpool", bufs=6))
        epool = ctx.enter_context(tc.tile_pool(name="epool", bufs=6))
        tbpool = ctx.enter_context(tc.tile_pool(name="tbpool", bufs=4))
        rbpool = ctx.enter_context(tc.tile_pool(name="rbpool", bufs=4))
        opool = ctx.enter_context(tc.tile_pool(name="opool", bufs=4))
        scp = ctx.enter_context(tc.tile_pool(name="scp", bufs=1, space="PSUM"))
        tp = ctx.enter_context(tc.tile_pool(name="tp", bufs=1, space="PSUM"))

        xts = [pool.tile([128, L], F16, name=f"xt{k}") for k in range(KD)]
        wq_sb = pool.tile([128, KD, FPC], F16)
        wk_sb = pool.tile([128, KD, FPC], F16)
        wv_sb = pool.tile([128, KD, FPC], F16)
        wo_sb = pool.tile([128, 2, D], F16)
        bq_sb = pool.tile([128, 2], F32)
        bk_sb = pool.tile([128, 2], F32)
        scratch = pool.tile([128, 128], F16)
        # first compute (V kd0) needs only xt0+wv: put those first so the
        # PE starts ~1us in and the HAM clock never re-throttles
        nc.sync.dma_start(out=xts[0][:], in_=xT[:, 0, :])
        nc.sync.dma_start(out=wv_sb[:], in_=wv[:])
        nc.sync.dma_start(out=wk_sb[:], in_=wk[:])
        nc.sync.dma_start(out=wq_sb[:], in_=wq[:])
        for kd in range(1, KD):
            nc.sync.dma_start(out=xts[kd][:], in_=xT[:, kd, :])
        nc.sync.dma_start(out=bk_sb[:], in_=bk[:])
        nc.sync.dma_start(out=bq_sb[:], in_=bq[:])
        nc.sync.dma_start(out=wo_sb[:], in_=wo[:])

        QT = pool.tile([128, 2, L], F16)   # [feat(2x128), l]: Q^T * 0.125
        KT = pool.tile([128, 2, L], F16)
        # Vaug[:, st, i, g]: head h = 2g+i; i=0 -> [V_h | 1], i=1 -> [1 | V_h]
        Vaug = pool.tile([128, ST, 2, 2, 128], F16)
        nc.vector.memset(scratch[:], 0.0)
        nc.vector.memset(Vaug[:], 1.0)
        outTs = [pool.tile([128, 2, LTW], F16, name=f"outT{i}")
                 for i in range(LT)]

        # ---- HAM warm-up: dummy matmuls during the DMA window
        warm = tp.tile([128, 128], F32, tag="T0", name="warm")
        for _ in range(24):
            nc.tensor.matmul(warm[:], scratch[:], scratch[:],
                             start=True, stop=True)

        def v_copies(st, psv):
            # even heads (i=0): av lanes -> cols 0:64; odd: cols 64:128
            nc.vector.tensor_copy(Vaug[:, st, 0, :, 0:DK], psv[:, 0])
            nc.vector.tensor_copy(Vaug[:, st, 1, :, DK:128], psv[:, 1])

        _inj = [0]

        def _next_inj_tag():
            _inj[0] ^= 1
            return "scA" if _inj[0] else "scB"

        def emit_kt_piece(c, ft, half):
            # N=256 piece: small PE burst; inject these two at a time to
            # keep the "sc" rotation parity
            lsl = slice(c * LTW + half * 256, c * LTW + (half + 1) * 256)
            ps = scp.tile([128, 256], F32, tag=_next_inj_tag(), name="pk")
            for kd in range(KD):
                nc.tensor.matmul(ps[:], wk_sb[:, kd, ft * 128:(ft + 1) * 128],
                                 xts[kd][:, lsl],
                                 start=(kd == 0), stop=(kd == KD - 1))
            nc.vector.scalar_tensor_tensor(
                KT[:, ft, lsl], ps[:], 1.0,
                bk_sb[:, ft:ft + 1].to_broadcast((128, 256)), MULT, ADD)

        def emit_qt_piece(lt, ft, half):
            lsl = slice(lt * LTW + half * 256, lt * LTW + (half + 1) * 256)
            ps = scp.tile([128, 256], F32, tag=_next_inj_tag(), name="pq")
            for kd in range(KD):
                nc.tensor.matmul(ps[:], wq_sb[:, kd, ft * 128:(ft + 1) * 128],
                                 xts[kd][:, lsl],
                                 start=(kd == 0), stop=(kd == KD - 1))
            nc.scalar.activation(QT[:, ft, lsl], ps[:], Ident,
                                 bias=bq_sb[:, ft:ft + 1], scale=0.125)

        # -------- prologue: V (all 16 s-tiles, 4 per T-bank group), KT
        # -------- chunk0 + QT0 kd-interleaved into the first group so the
        # -------- PE tracks the xT chunk DMAs --------
        for grp in range(4):
            psvs = [tp.tile([128, 2, 2, DK], F32, tag=f"T{s % 4}",
                            name=f"psv{s}")
                    for s in range(4 * grp, 4 * grp + 4)]
            if grp == 0:
                ktp = scp.tile([128, 2, LTW], F32, tag="scA", name="ktp")
                qtp = scp.tile([128, 2, LTW], F32, tag="scB", name="qtp")
            for kd in range(KD):
                st_, sp_ = (kd == 0), (kd == KD - 1)
                for s in range(4):
                    st = 4 * grp + s
                    nc.tensor.matmul(psvs[s][:],
                                     xts[kd][:, st * STW:(st + 1) * STW],
                                     wv_sb[:, kd, :], start=st_, stop=sp_)
                if grp == 0:
                    for ft in range(2):
                        fsl = slice(ft * 128, (ft + 1) * 128)
                        nc.tensor.matmul(ktp[:, ft, :], wk_sb[:, kd, fsl],
                                         xts[kd][:, 0:LTW],
                                         start=st_, stop=sp_)
                        nc.tensor.matmul(qtp[:, ft, :], wq_sb[:, kd, fsl],
                                         xts[kd][:, 0:LTW],
                                         start=st_, stop=sp_)

            for s in range(4):
                v_copies(4 * grp + s, psvs[s])
            if grp == 0:
                for ft in range(2):
                    nc.vector.scalar_tensor_tensor(
                        KT[:, ft, 0:LTW], ktp[:, ft, :], 1.0,
                        bk_sb[:, ft:ft + 1].to_broadcast((128, LTW)),
                        MULT, ADD)
                    nc.scalar.activation(QT[:, ft, 0:LTW], qtp[:, ft, :],
                                         Ident, bias=bq_sb[:, ft:ft + 1],
                                         scale=0.125)

        def epi_full(lt, Ts):
            # Ordered to minimize DVE FIFO stalls on the gpsimd lane-swaps:
            # bounce copies first (feed the i=0 swaps), then the i=1 recips
            # (read PSUM directly, no DMA dependency), then the DMA-fed
            # recips, then the multiplies in DMA-completion order.
            tb = {}
            rb = {}
            for h in (0, 2):   # i=0: sums at 64:128, DMA can't read PSUM
                t = tbpool.tile([128, LTW], F32, name="tsb")
                nc.vector.tensor_copy(t[DK:128, :], Ts[h][DK:128, :])
                tb[h] = t
            for h in range(HPC):
                rb[h] = rbpool.tile([128, LTW], F32, name="rb")
            for h in (1, 3):   # sums 0:64: recip straight from PSUM
                nc.vector.reciprocal_approx_fast(out=rb[h][0:DK, :],
                                                 in_=Ts[h][0:DK, :])
                nc.gpsimd.dma_start(out=rb[h][DK:128, :], in_=rb[h][0:DK, :])
            for h in (0, 2):
                nc.gpsimd.dma_start(out=rb[h][0:DK, :], in_=tb[h][DK:128, :])
                nc.vector.reciprocal_approx_fast(out=rb[h][0:DK, :],
                                                 in_=rb[h][0:DK, :])
            for h in (1, 3, 0, 2):
                g, i = divmod(h, 2)
                av_sl = slice(DK * i, DK * (i + 1))
                nc.vector.tensor_mul(outTs[lt][av_sl, g, :],
                                     Ts[h][av_sl, :], rb[h][av_sl, :])

        def emit_outproj(lt, g):
            lt8 = 4 * lt + g
            csl = slice(g * 128, (g + 1) * 128)
            for nf in range(2):
                nsl = slice(nf * LTW, (nf + 1) * LTW)
                ps3 = scp.tile([128, LTW], F32, tag=_next_inj_tag(),
                               name="ps3")
                for pair in range(2):
                    nc.tensor.matmul(ps3[:], outTs[lt][:, pair, csl],
                                     wo_sb[:, pair, nsl],
                                     start=(pair == 0), stop=(pair == 1))
                ob = opool.tile([128, LTW], F16)
                if nf == 0:
                    nc.scalar.copy(ob[:], ps3[:])
                    nc.gpsimd.dma_start(out=out[:, lt8, nf], in_=ob[:])
                else:
                    nc.vector.tensor_copy(ob[:], ps3[:])
                    nc.sync.dma_start(out=out[:, lt8, nf], in_=ob[:])

        # -------- fused attention loop --------
        prev = []  # queue of (st, Es) awaiting AV (2-iteration lag)
        prev_Ts = None

        def emit_av(Ts, pst, pEs):
            for pair in range(2):
                for i in range(2):
                    h = 2 * pair + i
                    nc.tensor.matmul(Ts[h][:], Vaug[:, pst, i, pair, :],
                                     pEs[pair][:, i, :],
                                     start=(pst == 0), stop=(pst == ST - 1))

        for lt in range(LT):
            lsl = slice(lt * LTW, (lt + 1) * LTW)
            Ts = [tp.tile([128, LTW], F32, tag=f"T{h}", name=f"T{h}_{lt}")
                  for h in range(HPC)]
            for st in range(ST):
                # ---- previous l-tile epilogue, ahead of the body so the
                # ---- T banks free before this l-tile's AVs need them ----
                if lt > 0 and st == 0:
                    epi_full(lt - 1, prev_Ts)
                # ---- scores / exp / mask, pair order alternating ----
                ssl = slice(st * STW, (st + 1) * STW)
                mk = mpool.tile([128, LTW], F16)
                nc.sync.dma_start(out=mk[:], in_=maskT[st, lt])
                Es = [None, None]
                order = (0, 1) if st % 2 == 0 else (1, 0)
                for j, pair in enumerate(order):
                    sc = scp.tile([128, 2, LTW], F32,
                                  tag="scA" if j == 0 else "scB")
                    for i in range(2):
                        nc.tensor.matmul(
                            sc[:, i, :],
                            KT[DK * i:DK * (i + 1), pair, ssl],
                            QT[DK * i:DK * (i + 1), pair, lsl],
                            start=True, stop=True)
                    E = epool.tile([128, 2, LTW], F16, name=f"E{pair}")
                    nc.scalar.activation(E[:], sc[:], Exp)
                    nc.vector.tensor_mul(
                        E[:], E[:],
                        mk[:, None, :].to_broadcast((128, 2, LTW)))
                    Es[pair] = E
                # ---- injected projection pieces (one per iteration) ----
                if lt == 0:
                    if st < 12:
                        emit_kt_piece(1 + st // 4, (st % 4) // 2, st % 2)
                    else:
                        emit_qt_piece(1, (st - 12) // 2, st % 2)
                elif lt < 3 and 2 <= st < 6:
                    emit_qt_piece(lt + 1, (st - 2) // 2, st % 2)
                if lt > 0 and st in (5, 7, 9, 11):
                    emit_outproj(lt - 1, (st - 5) // 2)
                # ---- AV, two iterations behind ----
                prev.append((st, Es))
                if len(prev) > 2:
                    pst, pEs = prev.pop(0)
                    emit_av(Ts, pst, pEs)
            for pst, pEs in prev:
                emit_av(Ts, pst, pEs)
            prev = []
            prev_Ts = Ts

        # -------- tail: lt3 epilogue straight out of PSUM --------
        epi_full(3, prev_Ts)
        for g in range(4):
            emit_outproj(3, g)

    nc.compile()
    return nc


def _get_nc():
    global _CACHED_NC
    if _CACHED_NC is None:
        _CACHED_NC = _build()
    return _CACHED_NC


def _prep_core_inputs(c, x, mask, Wq, bq, Wk, bk, Wv, Wo):
    b, g = divmod(c, 4)
    cs = slice(g * FPC, (g + 1) * FPC)

    xT = np.ascontiguousarray(
        x[b].T.reshape(KD, 128, L).transpose(1, 0, 2)).astype(np.float16)
    wq_c = np.ascontiguousarray(
        Wq[:, cs].reshape(KD, 128, FPC).transpose(1, 0, 2)).astype(np.float16)
    wk_c = np.ascontiguousarray(
        Wk[:, cs].reshape(KD, 128, FPC).transpose(1, 0, 2)).astype(np.float16)
    # wv columns permuted to [parity, pair, dk] so the Vaug copies batch as
    # two [128, 2, 64] strided copies per s-tile.
    wv_l = Wv[:, cs].reshape(D, 2, 2, DK).transpose(0, 2, 1, 3).reshape(D, FPC)
    wv_c = np.ascontiguousarray(
        wv_l.reshape(KD, 128, FPC).transpose(1, 0, 2)).astype(np.float16)
    wo_c = np.ascontiguousarray(
        Wo[cs, :].reshape(2, 128, D).transpose(1, 0, 2)).astype(np.float16)
    bq_c = np.ascontiguousarray(
        (bq[cs] * 0.125).reshape(2, 128).T).astype(np.float32)
    bk_c = np.ascontiguousarray(bk[cs].reshape(2, 128).T).astype(np.float32)
    mT = mask[b].astype(np.float16).T  # [S, L]
    maskT = np.ascontiguousarray(
        mT.reshape(ST, 128, LT, LTW).transpose(0, 2, 1, 3))
    return {"xT": xT, "wq": wq_c, "wk": wk_c, "wv": wv_c, "wo": wo_c,
            "bq": bq_c, "bk": bk_c, "maskT": maskT}


def kernel(x, mask, Wq, bq, Wk, bk, Wv, bv, Wo, bo):
    x = np.asarray(x, np.float32)
    mask = np.asarray(mask)
    Wq, bq = np.asarray(Wq, np.float32), np.asarray(bq, np.float32)
    Wk, bk = np.asarray(Wk, np.float32), np.asarray(bk, np.float32)
    Wv, bv = np.asarray(Wv, np.float32), np.asarray(bv, np.float32)
    Wo, bo = np.asarray(Wo, np.float32), np.asarray(bo, np.float32)

    nc = _get_nc()
    in_maps = [_prep_core_inputs(c, x, mask, Wq, bq, Wk, bk, Wv, Wo)
               for c in range(NCORES)]
    res = run_bass_kernel_spmd(nc, in_maps, list(range(NCORES)))

    const_vec = (bv @ Wo + bo).astype(np.float32)  # A rows sum to 1
    outs = []
    for b in range(B):
        acc = np.zeros((L, D), np.float32)
        for g in range(4):
            part = res.results[4 * b + g]["out"]  # [128, 16, 2, 512] fp16
            acc += part.reshape(128, ST, D).transpose(1, 0, 2).reshape(
                L, D).astype(np.float32)
        acc += const_vec
        outs.append(acc)
    return np.stack(outs)


# revision 23
# speedup vs baseline: 1.0331x; 1.0331x over previous
"""Multi-head attention (B=2, L=S=2048, D=1024, H=16) on 8 Trainium2 cores.

Sharding: core c -> batch b = c // 4, head group g = c % 4 (4 heads per core).
W_Q/K/V column-sharded (256 cols per core), W_O row-sharded (256 rows per core);
the 4 partial outputs per batch are summed on the host (plus bias terms).

Fully fused single-pass schedule.  The scalar-engine exp (~130us total) and
the PE stream work (~180us total) are the binding engines; the schedule keeps
the PE continuously fed:
  - ~2.5us of warm-up matmuls during the initial DMA window lift the PE
    clock gate (HAM) toward 2.4 GHz before the real work lands;
  - prologue computes V for all 16 s-tiles (grouped 4 per T-PSUM-bank, the
    first group kd-interleaved with KT chunk0 / QT0 so compute tracks the
    xT chunk DMAs); attention starts as soon as KT0/QT0/V are done;
  - KT chunks 1-3 and QT1..3 are injected as N=256 pieces, one per
    iteration, after the iteration body; the output projection of l-tile t
    runs during l-tile t+1; output DMAs alternate sync/gpsimd queues;
  - the inner loop emits AV(st-1) after scores/exp/mask(st) so the PE never
    head-of-line blocks on the current tile's mask-multiply;
  - per l-tile epilogue: the four T accumulators drain to SBUF at the next
    l-tile's first iteration (two copies on the scalar engine, two on the
    vector engine), then reciprocal / lane-swap / normalize run off the
    critical path; the tail (lt3) works straight out of PSUM.

Math per core (all matmul operands fp16, PSUM fp32):
  QT = 0.125*(x Wq + bq)^T, KT = (x Wk + bk)^T  (feature-major [256, L]);
  Vaug = [V_h | ones] (even h) / [ones | V_h] (odd h); bv folded out on the
  host (softmax rows sum to 1 => + bv @ Wo + bo once).
  S^T = KT^T QT per (pair, 128-key tile, 512-l tile) -- the two K=64 matmuls
  of a pair occupy disjoint PE row-groups and run concurrently;
  E = exp(S^T) * maskT;  T_h += Vaug_h^T E accumulates the head output AND
  the softmax row-sums (ones columns) in one full-array matmul;
  outT = T_av * recip(T_sum);  out_partial = outT^T Wo_rows.
"""
from contextlib import ExitStack

import numpy as np

import concourse.bass as bass
import concourse.mybir as mybir
import concourse.tile as tile
from concourse import bacc
from concourse.bass_utils import run_bass_kernel_spmd

F16 = mybir.dt.float16
F32 = mybir.dt.float32

D = 1024          # d_model
H = 16            # heads
DK = 64           # head dim
B, L = 2, 2048
NCORES = 8
HPC = 4           # heads per core
FPC = HPC * DK    # features per core = 256
KD = D // 128     # 8 contraction subtiles for projections
LT, LTW = 4, 512  # l tiles
ST, STW = 16, 128  # s tiles
Ident = mybir.ActivationFunctionType.Identity
Exp = mybir.ActivationFunctionType.Exp
MULT = mybir.AluOpType.mult
ADD = mybir.AluOpType.add

_CACHED_NC = None


def _build():
    nc = bacc.Bacc("TRN2", target_bir_lowering=False, debug=False,
                   num_devices=NCORES)
    xT = nc.declare_dram_parameter("xT", [128, KD, L], F16, isOutput=False)
    wq = nc.declare_dram_parameter("wq", [128, KD, FPC], F16, isOutput=False)
    wk = nc.declare_dram_parameter("wk", [128, KD, FPC], F16, isOutput=False)
    wv = nc.declare_dram_parameter("wv", [128, KD, FPC], F16, isOutput=False)
    wo = nc.declare_dram_parameter("wo", [128, 2, D], F16, isOutput=False)
    bq = nc.declare_dram_parameter("bq", [128, 2], F32, isOutput=False)
    bk = nc.declare_dram_parameter("bk", [128, 2], F32, isOutput=False)
    maskT = nc.declare_dram_parameter("maskT", [ST, LT, 128, LTW], F16,
                                      isOutput=False)
    out = nc.declare_dram_parameter("out", [128, ST, 2, LTW], F16,
                                    isOutput=True)

    with tile.TileContext(nc) as tc, ExitStack() as ctx:
        pool = ctx.enter_context(tc.tile_pool(name="pers", bufs=1))
        mpool = ctx.enter_context(tc.tile_pool(name="mpool", bufs=4))
        epool = ctx.enter_context(tc.tile_pool(name="epool", bufs=4))
        tbpool = ctx.enter_context(tc.tile_pool(name="tbpool", bufs=4))
        rbpool = ctx.enter_context(tc.tile_pool(name="rbpool", bufs=4))
        opool = ctx.enter_context(tc.tile_pool(name="opool", bufs=3))
        scp = ctx.enter_context(tc.tile_pool(name="scp", bufs=2, space="PSUM"))
        tp = ctx.enter_context(tc.tile_pool(name="tp", bufs=1, space="PSUM"))

        xt = pool.tile([128, KD, L], F16)
        wq_sb = pool.tile([128, KD, FPC], F16)
        wk_sb = pool.tile([128, KD, FPC], F16)
        wv_sb = pool.tile([128, KD, FPC], F16)
        wo_sb = pool.tile([128, 2, D], F16)
        bq_sb = pool.tile([128, 2], F32)
        bk_sb = pool.tile([128, 2], F32)
        scratch = pool.tile([128, 128], F16)
        nc.sync.dma_start(out=wv_sb[:], in_=wv[:])
        nc.sync.dma_start(out=wk_sb[:], in_=wk[:])
        for kd in range(KD):
            nc.sync.dma_start(out=xt[:, kd, :], in_=xT[:, kd, :])
        nc.sync.dma_start(out=bk_sb[:], in_=bk[:])
        nc.sync.dma_start(out=bq_sb[:], in_=bq[:])
        nc.sync.dma_start(out=wq_sb[:], in_=wq[:])
        nc.sync.dma_start(out=wo_sb[:], in_=wo[:])

        QT = pool.tile([128, 2, L], F16)   # [feat(2x128), l]: Q^T * 0.125
        KT = pool.tile([128, 2, L], F16)
        # Vaug[:, st, i, g]: head h = 2g+i; i=0 -> [V_h | 1], i=1 -> [1 | V_h]
        Vaug = pool.tile([128, ST, 2, 2, 128], F16)
        nc.vector.memset(scratch[:], 0.0)
        nc.vector.memset(Vaug[:], 1.0)
        outTs = [pool.tile([128, 2, LTW], F16, name=f"outT{i}")
                 for i in range(LT)]

        # ---- HAM warm-up: dummy matmuls during the DMA window
        warm = tp.tile([128, 128], F32, tag="T0", name="warm")
        for _ in range(24):
            nc.tensor.matmul(warm[:], scratch[:], scratch[:],
                             start=True, stop=True)

        def v_copies(st, psv):
            # even heads (i=0): av lanes -> cols 0:64; odd: cols 64:128
            nc.vector.tensor_copy(Vaug[:, st, 0, :, 0:DK], psv[:, 0])
            nc.vector.tensor_copy(Vaug[:, st, 1, :, DK:128], psv[:, 1])

        def emit_kt_piece(c, ft, half):
            # N=256 piece: small PE burst that hides inside one iteration
            lsl = slice(c * LTW + half * 256, c * LTW + (half + 1) * 256)
            ps = scp.tile([128, 256], F32, tag="sc", name="pk")
            for kd in range(KD):
                nc.tensor.matmul(ps[:], wk_sb[:, kd, ft * 128:(ft + 1) * 128],
                                 xt[:, kd, lsl],
                                 start=(kd == 0), stop=(kd == KD - 1))
            nc.vector.scalar_tensor_tensor(
                KT[:, ft, lsl], ps[:], 1.0,
                bk_sb[:, ft:ft + 1].to_broadcast((128, 256)), MULT, ADD)

        def emit_qt_piece(lt, ft, half):
            lsl = slice(lt * LTW + half * 256, lt * LTW + (half + 1) * 256)
            ps = scp.tile([128, 256], F32, tag="sc", name="pq")
            for kd in range(KD):
                nc.tensor.matmul(ps[:], wq_sb[:, kd, ft * 128:(ft + 1) * 128],
                                 xt[:, kd, lsl],
                                 start=(kd == 0), stop=(kd == KD - 1))
            nc.scalar.activation(QT[:, ft, lsl], ps[:], Ident,
                                 bias=bq_sb[:, ft:ft + 1], scale=0.125)

        # -------- prologue: V (all 16 s-tiles, 4 per T-bank group), KT
        # -------- chunk0 + QT0 kd-interleaved into the first group so the
        # -------- PE tracks the xT chunk DMAs --------
        for grp in range(4):
            psvs = [tp.tile([128, 2, 2, DK], F32, tag=f"T{s % 4}",
                            name=f"psv{s}")
                    for s in range(4 * grp, 4 * grp + 4)]
            if grp == 0:
                ktp = scp.tile([128, 2, LTW], F32, tag="sc", name="ktp")
                qtp = scp.tile([128, 2, LTW], F32, tag="sc", name="qtp")
            for kd in range(KD):
                st_, sp_ = (kd == 0), (kd == KD - 1)
                for s in range(4):
                    st = 4 * grp + s
                    nc.tensor.matmul(psvs[s][:],
                                     xt[:, kd, st * STW:(st + 1) * STW],
                                     wv_sb[:, kd, :], start=st_, stop=sp_)
                if grp == 0:
                    for ft in range(2):
                        fsl = slice(ft * 128, (ft + 1) * 128)
                        nc.tensor.matmul(ktp[:, ft, :], wk_sb[:, kd, fsl],
                                         xt[:, kd, 0:LTW],
                                         start=st_, stop=sp_)
                        nc.tensor.matmul(qtp[:, ft, :], wq_sb[:, kd, fsl],
                                         xt[:, kd, 0:LTW],
                                         start=st_, stop=sp_)
            for s in range(4):
                v_copies(4 * grp + s, psvs[s])
            if grp == 0:
                for ft in range(2):
                    nc.vector.scalar_tensor_tensor(
                        KT[:, ft, 0:LTW], ktp[:, ft, :], 1.0,
                        bk_sb[:, ft:ft + 1].to_broadcast((128, LTW)),
                        MULT, ADD)
                    nc.scalar.activation(QT[:, ft, 0:LTW], qtp[:, ft, :],
                                         Ident, bias=bq_sb[:, ft:ft + 1],
                                         scale=0.125)

        def epi_copy(lt, Ts, hs):
            tsb = []
            for h in hs:
                t = tbpool.tile([128, LTW], F32, name="tsb")
                # split across ACT and DVE so neither queue stalls the
                # next l-tile's first iterations
                if h < 2:
                    nc.scalar.copy(t[:], Ts[h][:])
                else:
                    nc.vector.tensor_copy(t[:], Ts[h][:])
                tsb.append(t)
            return tsb

        def epi_norm(lt, src, h, av_from_psum=None):
            # src: SBUF copy of Ts[h] (or PSUM tile when av_from_psum is it)
            g, i = divmod(h, 2)
            av_sl = slice(DK * i, DK * (i + 1))
            rs_sl = slice(DK * (1 - i), DK * (2 - i))
            av = av_from_psum if av_from_psum is not None else src
            rb = rbpool.tile([128, LTW], F32, name="rb")
            if i == 0:   # av 0:64, sums 64:128
                nc.gpsimd.dma_start(out=rb[0:DK, :], in_=src[rs_sl, :])
                nc.vector.reciprocal_approx_fast(out=rb[0:DK, :],
                                                 in_=rb[0:DK, :])
            else:        # sums 0:64 -> recip at base 0, then move up
                nc.vector.reciprocal_approx_fast(out=rb[0:DK, :],
                                                 in_=src[rs_sl, :])
                nc.gpsimd.dma_start(out=rb[DK:128, :], in_=rb[0:DK, :])
            nc.vector.tensor_mul(outTs[lt][av_sl, g, :],
                                 av[av_sl, :], rb[av_sl, :])

        def emit_outproj(lt, g):
            lt8 = 4 * lt + g
            csl = slice(g * 128, (g + 1) * 128)
            ps3 = scp.tile([128, 2, LTW], F32, tag="sc", name="ps3")
            for nf in range(2):
                nsl = slice(nf * LTW, (nf + 1) * LTW)
                for pair in range(2):
                    nc.tensor.matmul(ps3[:, nf, :], outTs[lt][:, pair, csl],
                                     wo_sb[:, pair, nsl],
                                     start=(pair == 0), stop=(pair == 1))
            ob = opool.tile([128, 2, LTW], F16)
            if lt8 % 2 == 0:
                nc.scalar.copy(ob[:], ps3[:])
                nc.gpsimd.dma_start(out=out[:, lt8], in_=ob[:])
            else:
                nc.vector.tensor_copy(ob[:], ps3[:])
                nc.sync.dma_start(out=out[:, lt8], in_=ob[:])

        # -------- fused attention loop --------
        prev = None
        ep_tsb = None
        prev_Ts = None
        for lt in range(LT):
            lsl = slice(lt * LTW, (lt + 1) * LTW)
            Ts = [tp.tile([128, LTW], F32, tag=f"T{h}", name=f"T{h}_{lt}")
                  for h in range(HPC)]
            for st in range(ST):
                # ---- scores / exp / mask for this st ----
                ssl = slice(st * STW, (st + 1) * STW)
                mk = mpool.tile([128, LTW], F16)
                nc.sync.dma_start(out=mk[:], in_=maskT[st, lt])
                Es = []
                for pair in range(2):
                    sc = scp.tile([128, 2, LTW], F32, tag="sc")
                    for i in range(2):
                        nc.tensor.matmul(
                            sc[:, i, :],
                            KT[DK * i:DK * (i + 1), pair, ssl],
                            QT[DK * i:DK * (i + 1), pair, lsl],
                            start=True, stop=True)
                    E = epool.tile([128, 2, LTW], F16, name=f"E{pair}")
                    nc.scalar.activation(E[:], sc[:], Exp)
                    nc.vector.tensor_mul(
                        E[:], E[:],
                        mk[:, None, :].to_broadcast((128, 2, LTW)))
                    Es.append(E)
                # ---- injected work (after the body so this iteration's
                # ---- scores are already in the PE queue) ----
                if lt == 0:
                    if st < 12:
                        emit_kt_piece(1 + st // 4, (st % 4) // 2, st % 2)
                    else:
                        emit_qt_piece(1, (st - 12) // 2, st % 2)
                elif lt < 3 and 2 <= st < 6:
                    emit_qt_piece(lt + 1, (st - 2) // 2, st % 2)
                if lt > 0:
                    if st == 0:
                        ep_tsb = epi_copy(lt - 1, prev_Ts, range(HPC))
                    elif st == 1:
                        epi_norm(lt - 1, ep_tsb[0], 0)
                        epi_norm(lt - 1, ep_tsb[1], 1)
                    elif st == 2:
                        epi_norm(lt - 1, ep_tsb[2], 2)
                        epi_norm(lt - 1, ep_tsb[3], 3)
                    elif st in (5, 7, 9, 11):
                        emit_outproj(lt - 1, (st - 5) // 2)
                # ---- AV of the previous s-tile ----
                if prev is not None:
                    pst, pEs = prev
                    for pair in range(2):
                        for i in range(2):
                            h = 2 * pair + i
                            nc.tensor.matmul(Ts[h][:],
                                             Vaug[:, pst, i, pair, :],
                                             pEs[pair][:, i, :],
                                             start=(pst == 0),
                                             stop=(pst == ST - 1))
                prev = (st, Es)
            pst, pEs = prev
            for pair in range(2):
                for i in range(2):
                    h = 2 * pair + i
                    nc.tensor.matmul(Ts[h][:], Vaug[:, pst, i, pair, :],
                                     pEs[pair][:, i, :],
                                     start=(pst == 0), stop=(pst == ST - 1))
            prev = None
            prev_Ts = Ts

        # -------- tail: lt3 epilogue straight out of PSUM --------
        for h in range(HPC):
            g, i = divmod(h, 2)
            if i == 0:
                # gpsimd DMA cannot read PSUM: bounce the sums via SBUF
                t = tbpool.tile([128, LTW], F32, name="tsb")
                nc.vector.tensor_copy(t[DK:128, :], prev_Ts[h][DK:128, :])
                epi_norm(3, t, h, av_from_psum=prev_Ts[h])
            else:
                epi_norm(3, prev_Ts[h], h, av_from_psum=prev_Ts[h])
        for g in range(4):
            emit_outproj(3, g)

    nc.compile()
    return nc


def _get_nc():
    global _CACHED_NC
    if _CACHED_NC is None:
        _CACHED_NC = _build()
    return _CACHED_NC


def _prep_core_inputs(c, x, mask, Wq, bq, Wk, bk, Wv, Wo):
    b, g = divmod(c, 4)
    cs = slice(g * FPC, (g + 1) * FPC)

    xT = np.ascontiguousarray(
        x[b].T.reshape(KD, 128, L).transpose(1, 0, 2)).astype(np.float16)
    wq_c = np.ascontiguousarray(
        Wq[:, cs].reshape(KD, 128, FPC).transpose(1, 0, 2)).astype(np.float16)
    wk_c = np.ascontiguousarray(
        Wk[:, cs].reshape(KD, 128, FPC).transpose(1, 0, 2)).astype(np.float16)
    # wv columns permuted to [parity, pair, dk] so the Vaug copies batch as
    # two [128, 2, 64] strided copies per s-tile.
    wv_l = Wv[:, cs].reshape(D, 2, 2, DK).transpose(0, 2, 1, 3).reshape(D, FPC)
    wv_c = np.ascontiguousarray(
        wv_l.reshape(KD, 128, FPC).transpose(1, 0, 2)).astype(np.float16)
    wo_c = np.ascontiguousarray(
        Wo[cs, :].reshape(2, 128, D).transpose(1, 0, 2)).astype(np.float16)
    bq_c = np.ascontiguousarray(
        (bq[cs] * 0.125).reshape(2, 128).T).astype(np.float32)
    bk_c = np.ascontiguousarray(bk[cs].reshape(2, 128).T).astype(np.float32)
    mT = mask[b].astype(np.float16).T  # [S, L]
    maskT = np.ascontiguousarray(
        mT.reshape(ST, 128, LT, LTW).transpose(0, 2, 1, 3))
    return {"xT": xT, "wq": wq_c, "wk": wk_c, "wv": wv_c, "wo": wo_c,
            "bq": bq_c, "bk": bk_c, "maskT": maskT}


def kernel(x, mask, Wq, bq, Wk, bk, Wv, bv, Wo, bo):
    x = np.asarray(x, np.float32)
    mask = np.asarray(mask)
    Wq, bq = np.asarray(Wq, np.float32), np.asarray(bq, np.float32)
    Wk, bk = np.asarray(Wk, np.float32), np.asarray(bk, np.float32)
    Wv, bv = np.asarray(Wv, np.float32), np.asarray(bv, np.float32)
    Wo, bo = np.asarray(Wo, np.float32), np.asarray(bo, np.float32)

    nc = _get_nc()
    in_maps = [_prep_core_inputs(c, x, mask, Wq, bq, Wk, bk, Wv, Wo)
               for c in range(NCORES)]
    res = run_bass_kernel_spmd(nc, in_maps, list(range(NCORES)))

    const_vec = (bv @ Wo + bo).astype(np.float32)  # A rows sum to 1
    outs = []
    for b in range(B):
        acc = np.zeros((L, D), np.float32)
        for g in range(4):
            part = res.results[4 * b + g]["out"]  # [128, 16, 2, 512] fp16
            acc += part.reshape(128, ST, D).transpose(1, 0, 2).reshape(
                L, D).astype(np.float32)
        acc += const_vec
        outs.append(acc)
    return np.stack(outs)
